# revision 17
# baseline (speedup 1.0000x reference)
"""Multi-headed self-attention (S=2048, D=1024, H=16) on 8 trn2 NeuronCores.

Sharding: tensor-parallel over heads (2 heads/core), fully collective-free.
Each core computes q/k/v for its 2 heads, runs base-2 no-max softmax
attention, and then computes the PARTIAL output projection for the full
[S, D] output (K-split over heads). The host unshard step sums the 8
partial outputs.

v3 (from HW trace analysis of v2 @156.8us):
- Pass order k -> q(chunk0) -> attention; v-pass, q(chunk1) and the v'
  transposes interleave into attention(0,0)'s tt loop (q1 BEFORE the
  transposes: in v2 the in-order PE queue ran q1 ~15us late and stalled
  chunk-1 attention for 5.5us). ctx(0,0) defers into (0,1)'s loop,
  2 t-tiles per step so norm(0,0) lands mid-(0,1).
- The exp softmax is SPLIT across engines: the Activation engine
  sustains only ~1.33us per [128,1024] exp tile under its activity
  throttle, so in the windows where it is the bottleneck a subset of
  t-tiles is computed on the DVE via two custom table ops:
  g = t(a1+t(a2+t a3)) ~= 2^t - 1 (scores prescaled by 1/128 via kT2),
  then p = (1+g)^16. Max rel err 4.5e-3 (measured on HW), fro impact
  ~2e-4 -- the scores PSUM stays fp32, only p is fp16.
- Tail: single full-width (1,1) pass (v2's 512-split cost +6us of
  Activation time); normalize runs in two 512 halves and the 8 chunk-1
  projection blocks pipeline through a 3-psum ring (sc ring + flex)
  with casts alternating Activation/DVE (Activation is idle post-exp).
- PSUM: sc pool 2x[128,1024]f32, ctx pool 1x, flex pool 1x shared (in
  strict sequence) by q0/v0/v1/q1 qkv psums, v' transposes, ctx(0,1),
  proj(0) blocks, ctx(1,1), and every third tail proj block.
"""

import sys

import numpy as np

if "/opt/trn_rl_repo" not in sys.path:
    sys.path.insert(0, "/opt/trn_rl_repo")

S, D, A, H = 2048, 1024, 1024, 16
NCORES = 8
HPC = H // NCORES            # heads per core = 2
HD = A // H                  # head dim = 64
E = HPC * HD                 # local qkv rows = 128
ND = D // 128                # d tiles = 8
NT = S // 128                # t tiles = 16
LN2 = 0.6931471805599453
# kT2 is prescaled by 1/128 host-side => scores psum = qk/128 = s_hat/16
# Act path: p = exp(psum * 16 ln2) = 2^s_hat;  DVE path: p = (1+g(t))^16
EXP_SCALE = 16.0 * LN2

# minimax-ish fit of g(t) ~= 2^t - 1 on t in [-0.62, 0.62], weighted for
# relative error of (1+g)^16 (see fit in the build notes); max rel err of
# the composed p on |s_hat|<=9.9 is ~6.6e-3 in fp32, 4.5e-3 measured.
PA1, PA2, PA3 = 0.6935366256724811, 0.24282106648173085, 0.05415638145524527

NCH = 2                      # attention s-chunks
CH = S // NCH                # 1024
NB = CH // 128               # proj s-blocks per chunk = 8

_CACHE = {}


def _register_dve_ops():
    """Register the two custom DVE table ops used by the DVE exp path.
    Idempotent; sha computed at runtime against this container's lower()."""
    from concourse import dve_ops
    from concourse.dve_spec import (
        Spec, Src0, C0, C1, C2, lower, _has_src1 as has_src1, sq,
    )
    from concourse.dve_uop import DveOpSpec
    from concourse.dve_table_gen import dve_ver_for

    have = {op.name for op in dve_ops.OPS}

    def register(name, body, ref):
        if name in have:
            return next(op for op in dve_ops.OPS if op.name == name)
        ver = dve_ver_for("TRN2")
        sp = Spec(body=body, reference=ref)
        row = dve_ops._CUSTOM_DVE_ROW_BASE + len(dve_ops.OPS)
        probe = DveOpSpec(name=name, opcode=row, uops=lower(sp, ver=ver),
                          rd1_en=has_src1(sp))
        op = dve_ops.DveOp(name, sp, subdim=False,
                           uops_sha={ver: probe.sha(ver)})
        dve_ops.OPS.append(op)
        dve_ops.CUSTOM_DVE_SPECS[name] = sp
        dve_ops._SUB_OPCODE_FOR_NAME[name] = row
        return op

    op_poly = register(
        "EXP2G_POLY_ANT",
        Src0 * (C0 + Src0 * (C1 + Src0 * C2)),
        lambda in0, in1, s0, s1, imm2: (
            in0.astype(np.float32) * (s0 + in0 * (s1 + in0 * imm2))),
    )
    op_pow16 = register(
        "EXP2G_POW16_ANT",
        sq(sq(sq(sq(Src0 + C0)))),
        lambda in0, in1, s0, s1, imm2: ((in0.astype(np.float32) + s0) ** 16),
    )
    return op_poly, op_pow16


def _build(enable_asserts=False):
    import concourse.bass as bass
    import concourse.tile as tile
    import concourse.mybir as mybir
    from concourse import bacc
    from concourse.masks import make_identity

    dve_exp_ops = _register_dve_ops()

    f16 = mybir.dt.float16

    nc = bacc.Bacc(
        "TRN2",
        target_bir_lowering=False,
        debug=False,
        enable_asserts=enable_asserts,
        num_devices=NCORES,
    )

    xT = nc.dram_tensor("xT", [ND, 128, S], f16, kind="ExternalInput").ap()
    wk = nc.dram_tensor("wk", [128, ND * E], f16, kind="ExternalInput").ap()
    wv = nc.dram_tensor("wv", [128, ND * E], f16, kind="ExternalInput").ap()
    wq = nc.dram_tensor("wq", [128, ND * E], f16, kind="ExternalInput").ap()
    wol = nc.dram_tensor("wol", [128, D], f16, kind="ExternalInput").ap()
    out = nc.dram_tensor("out", [NCH, NB, 128, D], f16, kind="ExternalOutput").ap()

    with tile.TileContext(nc) as tc:
        _body(tc, xT, (wk, wv, wq), wol, out, mybir, bass, make_identity,
              dve_exp_ops)

    nc.compile()
    return nc


def _body(tc, xT, wkvq, wol, out, mybir, bass, make_identity, dve_exp_ops):
    from contextlib import ExitStack

    nc = tc.nc
    f16 = mybir.dt.float16
    f32 = mybir.dt.float32
    Exp = mybir.ActivationFunctionType.Exp
    op_poly, op_pow16 = dve_exp_ops

    ctx_stack = ExitStack()
    persist = ctx_stack.enter_context(tc.tile_pool(name="persist", bufs=1))

    def ptile(shape, dtype, name):
        return persist.tile(shape, dtype, tag=name, name=name)

    xt_g = [ptile([128, S], f16, f"xt_g{g}") for g in range(ND)]
    w_sb = [ptile([128, ND, E], f16, f"w_sb{i}") for i in range(3)]  # k,v,q
    wol_sb = ptile([128, D], f16, "wol_sb")
    qT_c = [ptile([128, CH], f16, f"qT_c{ci}") for ci in range(NCH)]
    kT2_sb = [ptile([128, S], f16, f"kT2_sb{h}") for h in range(HPC)]
    vT_sb = ptile([128, S], f16, "vT_sb")
    vp_sb = ptile([128, NT, 2 * (HD + 1)], f16, "vp_sb")
    ident_sb = ptile([128, 128], f16, "ident_sb")
    ctxn_sb = ptile([128, S], f16, "ctxn_sb")
    junk_sb = ptile([128, 512], f16, "junk_sb")

    nc.vector.memset(kT2_sb[0][HD:128, :], 0.0)
    nc.vector.memset(kT2_sb[1][0:HD, :], 0.0)
    nc.vector.memset(vp_sb[:, :, HD:HD + 1], 1.0)
    nc.vector.memset(vp_sb[:, :, 2 * HD + 1:2 * HD + 2], 1.0)
    nc.vector.memset(junk_sb[:], 0.0)
    make_identity(nc, ident_sb[:])

    nc.sync.dma_start(w_sb[0][:], wkvq[0].rearrange("p (t c) -> p t c", t=ND))
    for g in range(ND):
        eng = nc.scalar if g % 2 == 0 else nc.sync
        eng.dma_start(xt_g[g][:], xT[g])
    nc.scalar.dma_start(w_sb[1][:], wkvq[1].rearrange("p (t c) -> p t c", t=ND))
    nc.scalar.dma_start(w_sb[2][:], wkvq[2].rearrange("p (t c) -> p t c", t=ND))
    nc.scalar.dma_start(wol_sb[:], wol)

    with (
        tc.tile_pool(name="sc_ps", bufs=2, space="PSUM") as sc_ps,
        tc.tile_pool(name="ctx_ps", bufs=1, space="PSUM") as ctx_ps,
        tc.tile_pool(name="flex_ps", bufs=1, space="PSUM") as flex_ps,
        tc.tile_pool(name="pt_pool", bufs=22) as pt_pool,
        tc.tile_pool(name="g_pool", bufs=2) as g_pool,
        tc.tile_pool(name="nrm_pool", bufs=2) as nrm_pool,
        tc.tile_pool(name="out_pool", bufs=3) as out_pool,
    ):
        def sc_tile(name="sc"):
            return sc_ps.tile([128, CH], f32, tag="sc", name=name)

        def flex_tile(shape, dtype, name):
            return flex_ps.tile(shape, dtype, tag="flex", name=name)

        def qkv_mm(pss, wi, cols0, dts):
            for dt_ in dts:
                for nn in range(2):
                    nc.tensor.matmul(
                        pss[:, nn * 512:(nn + 1) * 512],
                        lhsT=w_sb[wi][:, dt_, :],
                        rhs=xt_g[dt_][:, cols0 + nn * 512:cols0 + (nn + 1) * 512],
                        start=(dt_ == 0),
                        stop=(dt_ == ND - 1),
                    )

        def k_copy(pss, c0, hf):
            # one 512-col half: kT2[0] rows on Act (idle pre-exp), kT2[1]
            # rows on DVE; halves emitted low-cols-first across both tiles
            cols = slice(c0 + hf * 512, c0 + (hf + 1) * 512)
            nc.scalar.copy(kT2_sb[0][0:HD, cols],
                           pss[0:HD, hf * 512:(hf + 1) * 512])
            nc.vector.tensor_copy(kT2_sb[1][HD:128, cols],
                                  pss[HD:128, hf * 512:(hf + 1) * 512])

        # ---- k (both halves) + q chunk-0, interleaved per d-tile ----
        kp = [sc_tile("k0"), sc_tile("k1")]
        q0 = flex_tile([128, CH], f32, "q0")
        jk = ctx_ps.tile([128, 512], f32, tag="ctx", name="jk")
        for dt_ in range(ND):
            for ti in range(2):
                qkv_mm(kp[ti], 0, ti * CH, [dt_])
            qkv_mm(q0, 2, 0, [dt_])
            # clock-ramp filler: the k-pass is DMA-gated, and idle gaps
            # hold the PE at the 1.2GHz p-state; junk matmuls keep it
            # "continuously executing" so the ramp to 2.4GHz completes
            # during the load instead of mid-attention
            if dt_ < 6:
                for _ in range(3):
                    nc.tensor.matmul(
                        jk[:], lhsT=junk_sb[:, 0:128], rhs=junk_sb[:],
                        start=True, stop=True,
                    )
        # q0 copies first: the first scores tile needs ALL of qT_c[0] but
        # only kT2 cols 0:128, so q0 must not queue behind 4 k copies
        nc.scalar.copy(qT_c[0][:, 0:512], q0[:, 0:512])
        nc.vector.tensor_copy(qT_c[0][:, 512:CH], q0[:, 512:CH])
        for hf in range(2):
            for ti in range(2):
                k_copy(kp[ti], ti * CH, hf)

        # ---- deferred front work run inside attention(0,0)'s loop; flex
        # sequence: q0, v0, v1, q1, transposes, ctx(0,1), proj(0), ctx(1,1)
        vp_tiles = [None, None]

        def v_mm(ti, dts):
            if vp_tiles[ti] is None:
                vp_tiles[ti] = flex_tile([128, CH], f32, f"v{ti}")
            qkv_mm(vp_tiles[ti], 1, ti * CH, dts)

        def v_copy(ti):
            nc.vector.tensor_copy(
                vT_sb[:, ti * CH:(ti + 1) * CH], vp_tiles[ti][:]
            )

        q1_tile = [None]

        def q1_mm(dts):
            if q1_tile[0] is None:
                q1_tile[0] = flex_tile([128, CH], f32, "q1")
            qkv_mm(q1_tile[0], 2, CH, dts)

        def transposes(tts):
            for tt in tts:
                tp = flex_tile([128, 128], f16, "tr")
                nc.tensor.transpose(
                    tp[:], vT_sb[:, tt * 128:(tt + 1) * 128], ident_sb[:]
                )
                # one strided copy for both heads' 64 columns (the ones
                # columns at HD and 2HD+1 are skipped by the 65-stride)
                nc.vector.tensor_copy(
                    vp_sb[:, tt].rearrange("p (h c) -> p h c", h=2)[:, :, 0:HD],
                    tp[:].rearrange("p (h c) -> p h c", h=2),
                )

        front_hooks = {
            1: lambda: v_mm(0, range(0, 4)),
            2: lambda: (v_mm(0, range(4, 8)), v_copy(0)),
            3: lambda: v_mm(1, range(0, 4)),
            4: lambda: (v_mm(1, range(4, 8)), v_copy(1)),
            6: lambda: q1_mm(range(0, 4)),
            7: lambda: q1_mm(range(4, 8)),
            8: lambda: nc.vector.tensor_copy(qT_c[1][:], q1_tile[0][:]),
            9: lambda: transposes(range(0, 4)),
            10: lambda: transposes(range(4, 8)),
            11: lambda: transposes(range(8, 12)),
            12: lambda: transposes(range(12, 16)),
        }

        def attn(ci, h, cpool, ctag, pts_out=None, interleave=None,
                 dve_tts=()):
            """Scores+exp for NT t-tiles; ctx software-pipelined (tt-2)
            unless deferred via pts_out. Tiles in dve_tts compute exp on
            the DVE (poly + pow16 custom ops) instead of Activation."""
            cx = cpool.tile([HD + 1, CH], f32, tag=ctag, name="ctx")

            def ctx_mm(tt, pt):
                for nn in range(2):
                    nc.tensor.matmul(
                        cx[:, nn * 512:(nn + 1) * 512],
                        lhsT=vp_sb[:, tt, h * (HD + 1):(h + 1) * (HD + 1)],
                        rhs=pt[:, nn * 512:(nn + 1) * 512],
                        start=(tt == 0),
                        stop=(tt == NT - 1),
                    )

            pts = {}
            for tt in range(NT):
                sc = sc_ps.tile([128, CH], f32, tag="sc", name="sc")
                for nn in range(2):
                    nc.tensor.matmul(
                        sc[:, nn * 512:(nn + 1) * 512],
                        lhsT=kT2_sb[h][:, tt * 128:(tt + 1) * 128],
                        rhs=qT_c[ci][:, nn * 512:(nn + 1) * 512],
                        start=True,
                        stop=True,
                    )
                pt = pt_pool.tile([128, CH], f16, tag="pt")
                if tt in dve_tts:
                    g = g_pool.tile([128, CH], f16, tag="g", name="g")
                    nc.vector._custom_dve(op_poly, out=g[:], in0=sc[:],
                                          s0=PA1, s1=PA2, imm2=PA3)
                    nc.vector._custom_dve(op_pow16, out=pt[:], in0=g[:],
                                          s0=1.0)
                else:
                    nc.scalar.activation(pt[:], sc[:], Exp, scale=EXP_SCALE)
                if pts_out is not None:
                    pts_out.append(pt)
                else:
                    pts[tt] = pt
                    if tt >= 2:
                        ctx_mm(tt - 2, pts.pop(tt - 2))
                if interleave is not None and tt in interleave:
                    interleave[tt]()
            if pts_out is None:
                ctx_mm(NT - 2, pts.pop(NT - 2))
                ctx_mm(NT - 1, pts.pop(NT - 1))
            return cx, ctx_mm

        def norm(ci, h, soff, W, cx):
            # the custom DVE recip cannot read PSUM (BITWISE_NOT seed reads
            # garbage through the PSUM port) -- copy the denominator first
            den = nrm_pool.tile([1, W], f32, tag=f"den{W}", name="den")
            nc.vector.tensor_copy(den[:], cx[HD:HD + 1, soff:soff + W])
            rec = nrm_pool.tile([1, W], f32, tag=f"rec{W}", name="rec")
            nc.vector.reciprocal_approx_fast(rec[:], den[:])
            rb = nrm_pool.tile([HD, W], f32, tag=f"rb{W}", name="rb")
            nc.gpsimd.partition_broadcast(rb[:], rec[:])
            nc.vector.tensor_mul(
                ctxn_sb[h * HD:(h + 1) * HD, ci * CH + soff:ci * CH + soff + W],
                cx[0:HD, soff:soff + W],
                rb[:],
            )

        def proj_block(ci, b, cast_engs, pool):
            # cast in 512-halves right after each matmul half; with subtile
            # dep tracking the next block's first matmul only WARs the
            # first half-cast, halving the psum-ring serialization
            ob = out_pool.tile([128, D], f16, tag="ob", name="ob")
            if pool is sc_ps:
                ps = sc_tile("proj")
            elif pool is ctx_ps:
                ps = ctx_ps.tile([128, CH], f32, tag="ctx", name="proj")
            else:
                ps = flex_tile([128, CH], f32, "proj")
            for nn in range(2):
                nc.tensor.matmul(
                    ps[:, nn * 512:(nn + 1) * 512],
                    lhsT=ctxn_sb[:, ci * CH + b * 128:
                                 ci * CH + (b + 1) * 128],
                    rhs=wol_sb[:, nn * 512:(nn + 1) * 512],
                    start=True,
                    stop=True,
                )
            if cast_engs == 0:
                nc.scalar.copy(ob[:], ps[:])
            else:
                nc.vector.tensor_copy(ob[:], ps[:])
            nc.sync.dma_start(out[ci, b], ob[:])

        # (0,0): scores+exp only, front work interleaved, pts kept
        pts00 = []
        cx00, ctx00_mm = attn(0, 0, ctx_ps, "ctx",
                              pts_out=pts00, interleave=front_hooks)
        # (0,1): ctx(0,0) at 2 tiles/step in the first half so norm(0,0)
        # lands mid-window and (1,0)'s ctx psum frees early
        cx01, _ = attn(0, 1, flex_ps, "flex",
                       interleave={tt: (lambda tt=tt: (
                           ctx00_mm(2 * tt, pts00[2 * tt]),
                           ctx00_mm(2 * tt + 1, pts00[2 * tt + 1]),
                           norm(0, 0, 0, CH, cx00) if tt == 7 else None,
                       )) for tt in range(8)})
        # (1,0): norm(0,1) emitted at tt2 (late: keeps its DVE ops out of
        # the exp-critical DVE queue head), proj(0) blocks from tt4
        h10 = {2: lambda: norm(0, 1, 0, CH, cx01)}
        for i in range(NB):
            h10[4 + i] = (lambda i=i: proj_block(0, i, 1, flex_ps))
        cx10, _ = attn(1, 0, ctx_ps, "ctx", interleave=h10, dve_tts=(13,))
        # (1,1): DVE absorbs 4 exp tiles (Act sustains only ~1.1-1.3us per
        # exp tile under its activity throttle); norm(1,0) is consumed
        # only by the chunk-1 projections, so it emits late at tt12
        cx11, _ = attn(1, 1, flex_ps, "flex", dve_tts=(2, 6, 10, 13),
                       interleave={12: lambda: norm(1, 0, 0, CH, cx10)})
        # clock bridge over the tail normalize window
        for i in range(2):
            jp = sc_tile("junk")
            nc.tensor.matmul(
                jp[:, 0:512], lhsT=junk_sb[:, 0:128],
                rhs=junk_sb[:, 0:512], start=True, stop=True,
            )
        # tail: norm in 512 halves (chains interleaved), 8 proj blocks
        # through a 3-psum ring (sc, sc, flex), casts split Act/DVE per
        # block half (Act is idle after the last exp)
        norm(1, 1, 0, 512, cx11)
        norm(1, 1, 512, 512, cx11)
        for b in range(NB):
            pool = (sc_ps, sc_ps, flex_ps, ctx_ps)[b % 4]
            proj_block(1, b, b % 2, pool)

    ctx_stack.close()


def get_nc(enable_asserts=False):
    key = ("nc", enable_asserts)
    if key not in _CACHE:
        _CACHE[key] = _build(enable_asserts)
    return _CACHE[key]


def make_in_maps(x, w_in, w_out):
    x = np.asarray(x, dtype=np.float32)
    w_in = np.asarray(w_in, dtype=np.float32)
    w_out = np.asarray(w_out, dtype=np.float32)
    xT = np.ascontiguousarray(x.T).astype(np.float16).reshape(ND, 128, S)
    w_outT = np.ascontiguousarray(w_out.T).astype(np.float16)  # [A, D]
    in_maps = []
    for c in range(NCORES):
        r0 = c * E
        def pmaj(wt, scale=1.0):  # [D, E] -> p-major [128, ND*E]
            return np.ascontiguousarray(
                (wt * scale).reshape(ND, 128, E).transpose(1, 0, 2)
                .reshape(128, ND * E)
            ).astype(np.float16)
        wq = pmaj(w_in[r0:r0 + E].T)
        # kT2 prescale: scores psum = qk/128 = s_hat/16 (see EXP_SCALE)
        wk = pmaj(w_in[A + r0:A + r0 + E].T, scale=1.0 / 128.0)
        wv = pmaj(w_in[2 * A + r0:2 * A + r0 + E].T)
        wol = np.ascontiguousarray(w_outT[r0:r0 + E])  # [128, D]
        in_maps.append({"xT": xT, "wk": wk, "wv": wv, "wq": wq, "wol": wol})
    return in_maps


def assemble_out(results):
    """results[c]["out"] is [NCH, NB, 128, D] fp16 partials in s-block
    order; the unshard step sums the 8 cores' partial projections."""
    full = np.zeros((S, D), dtype=np.float32)
    for c in range(NCORES):
        o = results[c]["out"].astype(np.float32).reshape(S, D)
        full += o
    return full


def kernel(x, w_in, w_out, tgt_len=None, **kwargs):
    from concourse.bass_utils import run_bass_kernel_spmd

    nc = get_nc()
    in_maps = make_in_maps(x, w_in, w_out)
    res = run_bass_kernel_spmd(nc, in_maps, core_ids=list(range(NCORES)))
    return assemble_out(res.results)


# revision 18
# speedup vs baseline: 1.0166x; 1.0166x over previous
"""Multi-headed self-attention (S=2048, D=1024, H=16) on 8 trn2 NeuronCores.

Sharding: tensor-parallel over heads (2 heads/core), fully collective-free.
Each core computes q/k/v for its 2 heads, runs base-2 no-max softmax
attention, and then computes the PARTIAL output projection for the full
[S, D] output (K-split over heads). The host unshard step sums the 8
partial outputs.

v3 (from HW trace analysis of v2 @156.8us):
- Pass order k -> q(chunk0) -> attention; v-pass, q(chunk1) and the v'
  transposes interleave into attention(0,0)'s tt loop (q1 BEFORE the
  transposes: in v2 the in-order PE queue ran q1 ~15us late and stalled
  chunk-1 attention for 5.5us). ctx(0,0) defers into (0,1)'s loop,
  2 t-tiles per step so norm(0,0) lands mid-(0,1).
- The exp softmax is SPLIT across engines: the Activation engine
  sustains only ~1.33us per [128,1024] exp tile under its activity
  throttle, so in the windows where it is the bottleneck a subset of
  t-tiles is computed on the DVE via two custom table ops:
  g = t(a1+t(a2+t a3)) ~= 2^t - 1 (scores prescaled by 1/128 via kT2),
  then p = (1+g)^16. Max rel err 4.5e-3 (measured on HW), fro impact
  ~2e-4 -- the scores PSUM stays fp32, only p is fp16.
- Tail: single full-width (1,1) pass (v2's 512-split cost +6us of
  Activation time); normalize runs in two 512 halves and the 8 chunk-1
  projection blocks pipeline through a 3-psum ring (sc ring + flex)
  with casts alternating Activation/DVE (Activation is idle post-exp).
- PSUM: sc pool 2x[128,1024]f32, ctx pool 1x, flex pool 1x shared (in
  strict sequence) by q0/v0/v1/q1 qkv psums, v' transposes, ctx(0,1),
  proj(0) blocks, ctx(1,1), and every third tail proj block.
"""

import sys

import numpy as np

if "/opt/trn_rl_repo" not in sys.path:
    sys.path.insert(0, "/opt/trn_rl_repo")

S, D, A, H = 2048, 1024, 1024, 16
NCORES = 8
HPC = H // NCORES            # heads per core = 2
HD = A // H                  # head dim = 64
E = HPC * HD                 # local qkv rows = 128
ND = D // 128                # d tiles = 8
NT = S // 128                # t tiles = 16
LN2 = 0.6931471805599453
# kT2 is prescaled by 1/128 host-side => scores psum = qk/128 = s_hat/16
# Act path: p = exp(psum * 16 ln2) = 2^s_hat;  DVE path: p = (1+g(t))^16
EXP_SCALE = 16.0 * LN2

# minimax-ish fit of g(t) ~= 2^t - 1 on t in [-0.62, 0.62], weighted for
# relative error of (1+g)^16 (see fit in the build notes); max rel err of
# the composed p on |s_hat|<=9.9 is ~6.6e-3 in fp32, 4.5e-3 measured.
PA1, PA2, PA3 = 0.6935366256724811, 0.24282106648173085, 0.05415638145524527

NCH = 2                      # attention s-chunks
CH = S // NCH                # 1024
NB = CH // 128               # proj s-blocks per chunk = 8

_CACHE = {}


def _register_dve_ops():
    """Register the two custom DVE table ops used by the DVE exp path.
    Idempotent; sha computed at runtime against this container's lower()."""
    from concourse import dve_ops
    from concourse.dve_spec import (
        Spec, Src0, C0, C1, C2, lower, _has_src1 as has_src1, sq,
    )
    from concourse.dve_uop import DveOpSpec
    from concourse.dve_table_gen import dve_ver_for

    have = {op.name for op in dve_ops.OPS}

    def register(name, body, ref):
        if name in have:
            return next(op for op in dve_ops.OPS if op.name == name)
        ver = dve_ver_for("TRN2")
        sp = Spec(body=body, reference=ref)
        row = dve_ops._CUSTOM_DVE_ROW_BASE + len(dve_ops.OPS)
        probe = DveOpSpec(name=name, opcode=row, uops=lower(sp, ver=ver),
                          rd1_en=has_src1(sp))
        op = dve_ops.DveOp(name, sp, subdim=False,
                           uops_sha={ver: probe.sha(ver)})
        dve_ops.OPS.append(op)
        dve_ops.CUSTOM_DVE_SPECS[name] = sp
        dve_ops._SUB_OPCODE_FOR_NAME[name] = row
        return op

    op_poly = register(
        "EXP2G_POLY_ANT",
        Src0 * (C0 + Src0 * (C1 + Src0 * C2)),
        lambda in0, in1, s0, s1, imm2: (
            in0.astype(np.float32) * (s0 + in0 * (s1 + in0 * imm2))),
    )
    op_pow16 = register(
        "EXP2G_POW16_ANT",
        sq(sq(sq(sq(Src0 + C0)))),
        lambda in0, in1, s0, s1, imm2: ((in0.astype(np.float32) + s0) ** 16),
    )
    return op_poly, op_pow16


def _build(enable_asserts=False):
    import concourse.bass as bass
    import concourse.tile as tile
    import concourse.mybir as mybir
    from concourse import bacc
    from concourse.masks import make_identity

    dve_exp_ops = _register_dve_ops()

    f16 = mybir.dt.float16

    nc = bacc.Bacc(
        "TRN2",
        target_bir_lowering=False,
        debug=False,
        enable_asserts=enable_asserts,
        num_devices=NCORES,
    )

    xT = nc.dram_tensor("xT", [ND, 128, S], f16, kind="ExternalInput").ap()
    wk = nc.dram_tensor("wk", [128, ND * E], f16, kind="ExternalInput").ap()
    wv = nc.dram_tensor("wv", [128, ND * E], f16, kind="ExternalInput").ap()
    wq = nc.dram_tensor("wq", [128, ND * E], f16, kind="ExternalInput").ap()
    wol = nc.dram_tensor("wol", [128, D], f16, kind="ExternalInput").ap()
    out = nc.dram_tensor("out", [NCH, NB, 128, D], f16, kind="ExternalOutput").ap()

    with tile.TileContext(nc) as tc:
        _body(tc, xT, (wk, wv, wq), wol, out, mybir, bass, make_identity,
              dve_exp_ops)

    nc.compile()
    return nc


def _body(tc, xT, wkvq, wol, out, mybir, bass, make_identity, dve_exp_ops):
    from contextlib import ExitStack

    nc = tc.nc
    f16 = mybir.dt.float16
    f32 = mybir.dt.float32
    Exp = mybir.ActivationFunctionType.Exp
    op_poly, op_pow16 = dve_exp_ops

    ctx_stack = ExitStack()
    persist = ctx_stack.enter_context(tc.tile_pool(name="persist", bufs=1))

    def ptile(shape, dtype, name):
        return persist.tile(shape, dtype, tag=name, name=name)

    xt_g = [ptile([128, S], f16, f"xt_g{g}") for g in range(ND)]
    w_sb = [ptile([128, ND, E], f16, f"w_sb{i}") for i in range(3)]  # k,v,q
    wol_sb = ptile([128, D], f16, "wol_sb")
    qT_c = [ptile([128, CH], f16, f"qT_c{ci}") for ci in range(NCH)]
    kT2_sb = [ptile([128, S], f16, f"kT2_sb{h}") for h in range(HPC)]
    vT_sb = ptile([128, S], f16, "vT_sb")
    vp_sb = ptile([128, NT, 2 * (HD + 1)], f16, "vp_sb")
    ident_sb = ptile([128, 128], f16, "ident_sb")
    ctxn_sb = ptile([128, S], f16, "ctxn_sb")
    junk_sb = ptile([128, 512], f16, "junk_sb")

    nc.vector.memset(kT2_sb[0][HD:128, :], 0.0)
    nc.vector.memset(kT2_sb[1][0:HD, :], 0.0)
    nc.vector.memset(vp_sb[:, :, HD:HD + 1], 1.0)
    nc.vector.memset(vp_sb[:, :, 2 * HD + 1:2 * HD + 2], 1.0)
    nc.vector.memset(junk_sb[:], 0.0)
    make_identity(nc, ident_sb[:])

    nc.sync.dma_start(w_sb[0][:], wkvq[0].rearrange("p (t c) -> p t c", t=ND))
    for g in range(ND):
        eng = nc.scalar if g % 2 == 0 else nc.sync
        eng.dma_start(xt_g[g][:], xT[g])
    nc.scalar.dma_start(w_sb[1][:], wkvq[1].rearrange("p (t c) -> p t c", t=ND))
    nc.scalar.dma_start(w_sb[2][:], wkvq[2].rearrange("p (t c) -> p t c", t=ND))
    nc.scalar.dma_start(wol_sb[:], wol)

    with (
        tc.tile_pool(name="sc_ps", bufs=2, space="PSUM") as sc_ps,
        tc.tile_pool(name="ctx_ps", bufs=1, space="PSUM") as ctx_ps,
        tc.tile_pool(name="flex_ps", bufs=1, space="PSUM") as flex_ps,
        tc.tile_pool(name="pt_pool", bufs=22) as pt_pool,
        tc.tile_pool(name="g_pool", bufs=2) as g_pool,
        tc.tile_pool(name="nrm_pool", bufs=2) as nrm_pool,
        tc.tile_pool(name="out_pool", bufs=3) as out_pool,
    ):
        def sc_tile(name="sc"):
            return sc_ps.tile([128, CH], f32, tag="sc", name=name)

        def flex_tile(shape, dtype, name):
            return flex_ps.tile(shape, dtype, tag="flex", name=name)

        def qkv_mm(pss, wi, cols0, dts):
            for dt_ in dts:
                for nn in range(2):
                    nc.tensor.matmul(
                        pss[:, nn * 512:(nn + 1) * 512],
                        lhsT=w_sb[wi][:, dt_, :],
                        rhs=xt_g[dt_][:, cols0 + nn * 512:cols0 + (nn + 1) * 512],
                        start=(dt_ == 0),
                        stop=(dt_ == ND - 1),
                    )

        def k_copy(pss, c0, hf):
            # one 512-col half: kT2[0] rows on Act (idle pre-exp), kT2[1]
            # rows on DVE; halves emitted low-cols-first across both tiles
            cols = slice(c0 + hf * 512, c0 + (hf + 1) * 512)
            nc.scalar.copy(kT2_sb[0][0:HD, cols],
                           pss[0:HD, hf * 512:(hf + 1) * 512])
            nc.vector.tensor_copy(kT2_sb[1][HD:128, cols],
                                  pss[HD:128, hf * 512:(hf + 1) * 512])

        # ---- k (both halves) + q chunk-0, interleaved per d-tile ----
        kp = [sc_tile("k0"), sc_tile("k1")]
        q0 = flex_tile([128, CH], f32, "q0")
        for dt_ in range(ND):
            for ti in range(2):
                qkv_mm(kp[ti], 0, ti * CH, [dt_])
            qkv_mm(q0, 2, 0, [dt_])
        # q0 copies first: the first scores tile needs ALL of qT_c[0] but
        # only kT2 cols 0:128, so q0 must not queue behind 4 k copies
        nc.scalar.copy(qT_c[0][:, 0:512], q0[:, 0:512])
        nc.vector.tensor_copy(qT_c[0][:, 512:CH], q0[:, 512:CH])
        for hf in range(2):
            for ti in range(2):
                k_copy(kp[ti], ti * CH, hf)

        # ---- deferred front work run inside attention(0,0)'s loop; flex
        # sequence: q0, v0, v1, q1, transposes, ctx(0,1), proj(0), ctx(1,1)
        vp_tiles = [None, None]

        def v_mm(ti, dts):
            if vp_tiles[ti] is None:
                vp_tiles[ti] = flex_tile([128, CH], f32, f"v{ti}")
            qkv_mm(vp_tiles[ti], 1, ti * CH, dts)

        def v_copy(ti):
            nc.vector.tensor_copy(
                vT_sb[:, ti * CH:(ti + 1) * CH], vp_tiles[ti][:]
            )

        q1_tile = [None]

        def q1_mm(dts):
            if q1_tile[0] is None:
                q1_tile[0] = flex_tile([128, CH], f32, "q1")
            qkv_mm(q1_tile[0], 2, CH, dts)

        def transposes(tts):
            for tt in tts:
                tp = flex_tile([128, 128], f16, "tr")
                nc.tensor.transpose(
                    tp[:], vT_sb[:, tt * 128:(tt + 1) * 128], ident_sb[:]
                )
                # one strided copy for both heads' 64 columns (the ones
                # columns at HD and 2HD+1 are skipped by the 65-stride)
                nc.vector.tensor_copy(
                    vp_sb[:, tt].rearrange("p (h c) -> p h c", h=2)[:, :, 0:HD],
                    tp[:].rearrange("p (h c) -> p h c", h=2),
                )

        front_hooks = {
            1: lambda: v_mm(0, range(0, 4)),
            2: lambda: (v_mm(0, range(4, 8)), v_copy(0)),
            3: lambda: v_mm(1, range(0, 4)),
            4: lambda: (v_mm(1, range(4, 8)), v_copy(1)),
            6: lambda: q1_mm(range(0, 4)),
            7: lambda: q1_mm(range(4, 8)),
            8: lambda: nc.vector.tensor_copy(qT_c[1][:], q1_tile[0][:]),
            9: lambda: transposes(range(0, 4)),
            10: lambda: transposes(range(4, 8)),
            11: lambda: transposes(range(8, 12)),
            12: lambda: transposes(range(12, 16)),
        }

        def attn(ci, h, cpool, ctag, pts_out=None, interleave=None,
                 dve_tts=()):
            """Scores+exp for NT t-tiles; ctx software-pipelined (tt-2)
            unless deferred via pts_out. Tiles in dve_tts compute exp on
            the DVE (poly + pow16 custom ops) instead of Activation."""
            cx = cpool.tile([HD + 1, CH], f32, tag=ctag, name="ctx")

            def ctx_mm(tt, pt):
                for nn in range(2):
                    nc.tensor.matmul(
                        cx[:, nn * 512:(nn + 1) * 512],
                        lhsT=vp_sb[:, tt, h * (HD + 1):(h + 1) * (HD + 1)],
                        rhs=pt[:, nn * 512:(nn + 1) * 512],
                        start=(tt == 0),
                        stop=(tt == NT - 1),
                    )

            pts = {}
            for tt in range(NT):
                sc = sc_ps.tile([128, CH], f32, tag="sc", name="sc")
                for nn in range(2):
                    nc.tensor.matmul(
                        sc[:, nn * 512:(nn + 1) * 512],
                        lhsT=kT2_sb[h][:, tt * 128:(tt + 1) * 128],
                        rhs=qT_c[ci][:, nn * 512:(nn + 1) * 512],
                        start=True,
                        stop=True,
                    )
                pt = pt_pool.tile([128, CH], f16, tag="pt")
                if tt in dve_tts:
                    g = g_pool.tile([128, CH], f16, tag="g", name="g")
                    nc.vector._custom_dve(op_poly, out=g[:], in0=sc[:],
                                          s0=PA1, s1=PA2, imm2=PA3)
                    nc.vector._custom_dve(op_pow16, out=pt[:], in0=g[:],
                                          s0=1.0)
                else:
                    nc.scalar.activation(pt[:], sc[:], Exp, scale=EXP_SCALE)
                if pts_out is not None:
                    pts_out.append(pt)
                else:
                    pts[tt] = pt
                    if tt >= 2:
                        ctx_mm(tt - 2, pts.pop(tt - 2))
                if interleave is not None and tt in interleave:
                    interleave[tt]()
            if pts_out is None:
                ctx_mm(NT - 2, pts.pop(NT - 2))
                ctx_mm(NT - 1, pts.pop(NT - 1))
            return cx, ctx_mm

        def norm(ci, h, soff, W, cx):
            # the custom DVE recip cannot read PSUM (BITWISE_NOT seed reads
            # garbage through the PSUM port) -- copy the denominator first
            den = nrm_pool.tile([1, W], f32, tag=f"den{W}", name="den")
            nc.vector.tensor_copy(den[:], cx[HD:HD + 1, soff:soff + W])
            rec = nrm_pool.tile([1, W], f32, tag=f"rec{W}", name="rec")
            nc.vector.reciprocal_approx_fast(rec[:], den[:])
            rb = nrm_pool.tile([HD, W], f32, tag=f"rb{W}", name="rb")
            nc.gpsimd.partition_broadcast(rb[:], rec[:])
            nc.vector.tensor_mul(
                ctxn_sb[h * HD:(h + 1) * HD, ci * CH + soff:ci * CH + soff + W],
                cx[0:HD, soff:soff + W],
                rb[:],
            )

        def proj_block(ci, b, cast_engs, pool):
            # cast in 512-halves right after each matmul half; with subtile
            # dep tracking the next block's first matmul only WARs the
            # first half-cast, halving the psum-ring serialization
            ob = out_pool.tile([128, D], f16, tag="ob", name="ob")
            if pool is sc_ps:
                ps = sc_tile("proj")
            elif pool is ctx_ps:
                ps = ctx_ps.tile([128, CH], f32, tag="ctx", name="proj")
            else:
                ps = flex_tile([128, CH], f32, "proj")
            for nn in range(2):
                nc.tensor.matmul(
                    ps[:, nn * 512:(nn + 1) * 512],
                    lhsT=ctxn_sb[:, ci * CH + b * 128:
                                 ci * CH + (b + 1) * 128],
                    rhs=wol_sb[:, nn * 512:(nn + 1) * 512],
                    start=True,
                    stop=True,
                )
            if cast_engs == 0:
                nc.scalar.copy(ob[:], ps[:])
            else:
                nc.vector.tensor_copy(ob[:], ps[:])
            nc.sync.dma_start(out[ci, b], ob[:])

        # (0,0): scores+exp only, front work interleaved, pts kept
        pts00 = []
        cx00, ctx00_mm = attn(0, 0, ctx_ps, "ctx",
                              pts_out=pts00, interleave=front_hooks)
        # (0,1): ctx(0,0) at 2 tiles/step in the first half so norm(0,0)
        # lands mid-window and (1,0)'s ctx psum frees early
        cx01, _ = attn(0, 1, flex_ps, "flex",
                       interleave={tt: (lambda tt=tt: (
                           ctx00_mm(2 * tt, pts00[2 * tt]),
                           ctx00_mm(2 * tt + 1, pts00[2 * tt + 1]),
                           norm(0, 0, 0, CH, cx00) if tt == 7 else None,
                       )) for tt in range(8)})
        # (1,0): norm(0,1) emitted at tt2 (late: keeps its DVE ops out of
        # the exp-critical DVE queue head), proj(0) blocks from tt4
        h10 = {2: lambda: norm(0, 1, 0, CH, cx01)}
        for i in range(NB):
            h10[4 + i] = (lambda i=i: proj_block(0, i, 1, flex_ps))
        cx10, _ = attn(1, 0, ctx_ps, "ctx", interleave=h10, dve_tts=(13,))
        # (1,1): DVE absorbs 4 exp tiles (Act sustains only ~1.1-1.3us per
        # exp tile under its activity throttle); norm(1,0) is consumed
        # only by the chunk-1 projections, so it emits late at tt12
        cx11, _ = attn(1, 1, flex_ps, "flex", dve_tts=(2, 6, 10, 13),
                       interleave={12: lambda: norm(1, 0, 0, CH, cx10)})
        # clock bridge over the tail normalize window
        for i in range(2):
            jp = sc_tile("junk")
            nc.tensor.matmul(
                jp[:, 0:512], lhsT=junk_sb[:, 0:128],
                rhs=junk_sb[:, 0:512], start=True, stop=True,
            )
        # tail: norm in 512 halves (chains interleaved), 8 proj blocks
        # through a 3-psum ring (sc, sc, flex), casts split Act/DVE per
        # block half (Act is idle after the last exp)
        norm(1, 1, 0, 512, cx11)
        norm(1, 1, 512, 512, cx11)
        for b in range(NB):
            pool = (sc_ps, sc_ps, flex_ps, ctx_ps)[b % 4]
            proj_block(1, b, b % 2, pool)

    ctx_stack.close()


def get_nc(enable_asserts=False):
    key = ("nc", enable_asserts)
    if key not in _CACHE:
        _CACHE[key] = _build(enable_asserts)
    return _CACHE[key]


def make_in_maps(x, w_in, w_out):
    x = np.asarray(x, dtype=np.float32)
    w_in = np.asarray(w_in, dtype=np.float32)
    w_out = np.asarray(w_out, dtype=np.float32)
    xT = np.ascontiguousarray(x.T).astype(np.float16).reshape(ND, 128, S)
    w_outT = np.ascontiguousarray(w_out.T).astype(np.float16)  # [A, D]
    in_maps = []
    for c in range(NCORES):
        r0 = c * E
        def pmaj(wt, scale=1.0):  # [D, E] -> p-major [128, ND*E]
            return np.ascontiguousarray(
                (wt * scale).reshape(ND, 128, E).transpose(1, 0, 2)
                .reshape(128, ND * E)
            ).astype(np.float16)
        wq = pmaj(w_in[r0:r0 + E].T)
        # kT2 prescale: scores psum = qk/128 = s_hat/16 (see EXP_SCALE)
        wk = pmaj(w_in[A + r0:A + r0 + E].T, scale=1.0 / 128.0)
        wv = pmaj(w_in[2 * A + r0:2 * A + r0 + E].T)
        wol = np.ascontiguousarray(w_outT[r0:r0 + E])  # [128, D]
        in_maps.append({"xT": xT, "wk": wk, "wv": wv, "wq": wq, "wol": wol})
    return in_maps


def assemble_out(results):
    """results[c]["out"] is [NCH, NB, 128, D] fp16 partials in s-block
    order; the unshard step sums the 8 cores' partial projections."""
    full = np.zeros((S, D), dtype=np.float32)
    for c in range(NCORES):
        o = results[c]["out"].astype(np.float32).reshape(S, D)
        full += o
    return full


def kernel(x, w_in, w_out, tgt_len=None, **kwargs):
    from concourse.bass_utils import run_bass_kernel_spmd

    nc = get_nc()
    in_maps = make_in_maps(x, w_in, w_out)
    res = run_bass_kernel_spmd(nc, in_maps, core_ids=list(range(NCORES)))
    return assemble_out(res.results)


# revision 19
# speedup vs baseline: 1.0210x; 1.0044x over previous
"""Multi-headed self-attention (S=2048, D=1024, H=16) on 8 trn2 NeuronCores.

Sharding: tensor-parallel over heads (2 heads/core), fully collective-free.
Each core computes q/k/v for its 2 heads, runs base-2 no-max softmax
attention, and then computes the PARTIAL output projection for the full
[S, D] output (K-split over heads). The host unshard step sums the 8
partial outputs.

v3 (from HW trace analysis of v2 @156.8us):
- Pass order k -> q(chunk0) -> attention; v-pass, q(chunk1) and the v'
  transposes interleave into attention(0,0)'s tt loop (q1 BEFORE the
  transposes: in v2 the in-order PE queue ran q1 ~15us late and stalled
  chunk-1 attention for 5.5us). ctx(0,0) defers into (0,1)'s loop,
  2 t-tiles per step so norm(0,0) lands mid-(0,1).
- The exp softmax is SPLIT across engines: the Activation engine
  sustains only ~1.33us per [128,1024] exp tile under its activity
  throttle, so in the windows where it is the bottleneck a subset of
  t-tiles is computed on the DVE via two custom table ops:
  g = t(a1+t(a2+t a3)) ~= 2^t - 1 (scores prescaled by 1/128 via kT2),
  then p = (1+g)^16. Max rel err 4.5e-3 (measured on HW), fro impact
  ~2e-4 -- the scores PSUM stays fp32, only p is fp16.
- Tail: single full-width (1,1) pass (v2's 512-split cost +6us of
  Activation time); normalize runs in two 512 halves and the 8 chunk-1
  projection blocks pipeline through a 3-psum ring (sc ring + flex)
  with casts alternating Activation/DVE (Activation is idle post-exp).
- PSUM: sc pool 2x[128,1024]f32, ctx pool 1x, flex pool 1x shared (in
  strict sequence) by q0/v0/v1/q1 qkv psums, v' transposes, ctx(0,1),
  proj(0) blocks, ctx(1,1), and every third tail proj block.
"""

import sys

import numpy as np

if "/opt/trn_rl_repo" not in sys.path:
    sys.path.insert(0, "/opt/trn_rl_repo")

S, D, A, H = 2048, 1024, 1024, 16
NCORES = 8
HPC = H // NCORES            # heads per core = 2
HD = A // H                  # head dim = 64
E = HPC * HD                 # local qkv rows = 128
ND = D // 128                # d tiles = 8
NT = S // 128                # t tiles = 16
LN2 = 0.6931471805599453
# kT2 is prescaled by 1/128 host-side => scores psum = qk/128 = s_hat/16
# Act path: p = exp(psum * 16 ln2) = 2^s_hat;  DVE path: p = (1+g(t))^16
EXP_SCALE = 16.0 * LN2

# minimax-ish fit of g(t) ~= 2^t - 1 on t in [-0.62, 0.62], weighted for
# relative error of (1+g)^16 (see fit in the build notes); max rel err of
# the composed p on |s_hat|<=9.9 is ~6.6e-3 in fp32, 4.5e-3 measured.
PA1, PA2, PA3 = 0.6935366256724811, 0.24282106648173085, 0.05415638145524527

NCH = 2                      # attention s-chunks
CH = S // NCH                # 1024
NB = CH // 128               # proj s-blocks per chunk = 8

_CACHE = {}


def _register_dve_ops():
    """Register the two custom DVE table ops used by the DVE exp path.
    Idempotent; sha computed at runtime against this container's lower()."""
    from concourse import dve_ops
    from concourse.dve_spec import (
        Spec, Src0, C0, C1, C2, lower, _has_src1 as has_src1, sq,
    )
    from concourse.dve_uop import DveOpSpec
    from concourse.dve_table_gen import dve_ver_for

    have = {op.name for op in dve_ops.OPS}

    def register(name, body, ref):
        if name in have:
            return next(op for op in dve_ops.OPS if op.name == name)
        ver = dve_ver_for("TRN2")
        sp = Spec(body=body, reference=ref)
        row = dve_ops._CUSTOM_DVE_ROW_BASE + len(dve_ops.OPS)
        probe = DveOpSpec(name=name, opcode=row, uops=lower(sp, ver=ver),
                          rd1_en=has_src1(sp))
        op = dve_ops.DveOp(name, sp, subdim=False,
                           uops_sha={ver: probe.sha(ver)})
        dve_ops.OPS.append(op)
        dve_ops.CUSTOM_DVE_SPECS[name] = sp
        dve_ops._SUB_OPCODE_FOR_NAME[name] = row
        return op

    op_poly = register(
        "EXP2G_POLY_ANT",
        Src0 * (C0 + Src0 * (C1 + Src0 * C2)),
        lambda in0, in1, s0, s1, imm2: (
            in0.astype(np.float32) * (s0 + in0 * (s1 + in0 * imm2))),
    )
    op_pow16 = register(
        "EXP2G_POW16_ANT",
        sq(sq(sq(sq(Src0 + C0)))),
        lambda in0, in1, s0, s1, imm2: ((in0.astype(np.float32) + s0) ** 16),
    )
    # experimental: populate the perf-mode table slots for pow16 (fp16
    # SBUF->SBUF, eligible for 2X/4X) with the same uop program; seeded
    # through the compile cache since DveOp.compile only emits 1x
    ver = dve_ver_for("TRN2")
    spec16 = DveOpSpec(
        name=op_pow16.name,
        opcode=dve_ops.get_dve_sub_opcode(op_pow16.name),
        uops=lower(op_pow16.spec, ver=ver),
        rd1_en=False,
    )
    spec16.uops_2x = spec16.uops
    spec16.uops_2x_2p = spec16.uops
    spec16.uops_4x = spec16.uops
    dve_ops._COMPILE_CACHE[(op_pow16.name, ver)] = spec16
    return op_poly, op_pow16


def _build(enable_asserts=False):
    import concourse.bass as bass
    import concourse.tile as tile
    import concourse.mybir as mybir
    from concourse import bacc
    from concourse.masks import make_identity

    dve_exp_ops = _register_dve_ops()

    f16 = mybir.dt.float16

    nc = bacc.Bacc(
        "TRN2",
        target_bir_lowering=False,
        debug=False,
        enable_asserts=enable_asserts,
        num_devices=NCORES,
    )

    xT = nc.dram_tensor("xT", [ND, 128, S], f16, kind="ExternalInput").ap()
    wk = nc.dram_tensor("wk", [128, ND * E], f16, kind="ExternalInput").ap()
    wv = nc.dram_tensor("wv", [128, ND * E], f16, kind="ExternalInput").ap()
    wq = nc.dram_tensor("wq", [128, ND * E], f16, kind="ExternalInput").ap()
    wol = nc.dram_tensor("wol", [128, D], f16, kind="ExternalInput").ap()
    out = nc.dram_tensor("out", [NCH, NB, 128, D], f16, kind="ExternalOutput").ap()

    with tile.TileContext(nc) as tc:
        _body(tc, xT, (wk, wv, wq), wol, out, mybir, bass, make_identity,
              dve_exp_ops)

    nc.compile()
    return nc


def _body(tc, xT, wkvq, wol, out, mybir, bass, make_identity, dve_exp_ops):
    from contextlib import ExitStack

    nc = tc.nc
    f16 = mybir.dt.float16
    f32 = mybir.dt.float32
    Exp = mybir.ActivationFunctionType.Exp
    op_poly, op_pow16 = dve_exp_ops

    ctx_stack = ExitStack()
    persist = ctx_stack.enter_context(tc.tile_pool(name="persist", bufs=1))

    def ptile(shape, dtype, name):
        return persist.tile(shape, dtype, tag=name, name=name)

    xt_g = [ptile([128, S], f16, f"xt_g{g}") for g in range(ND)]
    w_sb = [ptile([128, ND, E], f16, f"w_sb{i}") for i in range(3)]  # k,v,q
    wol_sb = ptile([128, D], f16, "wol_sb")
    qT_c = [ptile([128, CH], f16, f"qT_c{ci}") for ci in range(NCH)]
    kT2_sb = [ptile([128, S], f16, f"kT2_sb{h}") for h in range(HPC)]
    vT_sb = ptile([128, S], f16, "vT_sb")
    vp_sb = ptile([128, NT, 2 * (HD + 1)], f16, "vp_sb")
    ident_sb = ptile([128, 128], f16, "ident_sb")
    ctxn_sb = ptile([128, S], f16, "ctxn_sb")
    junk_sb = ptile([128, 512], f16, "junk_sb")

    nc.vector.memset(kT2_sb[0][HD:128, :], 0.0)
    nc.vector.memset(kT2_sb[1][0:HD, :], 0.0)
    nc.vector.memset(vp_sb[:, :, HD:HD + 1], 1.0)
    nc.vector.memset(vp_sb[:, :, 2 * HD + 1:2 * HD + 2], 1.0)
    nc.vector.memset(junk_sb[:], 0.0)
    make_identity(nc, ident_sb[:])

    nc.sync.dma_start(w_sb[0][:], wkvq[0].rearrange("p (t c) -> p t c", t=ND))
    for g in range(ND):
        eng = nc.scalar if g % 2 == 0 else nc.sync
        eng.dma_start(xt_g[g][:], xT[g])
    nc.scalar.dma_start(w_sb[1][:], wkvq[1].rearrange("p (t c) -> p t c", t=ND))
    nc.scalar.dma_start(w_sb[2][:], wkvq[2].rearrange("p (t c) -> p t c", t=ND))
    nc.scalar.dma_start(wol_sb[:], wol)

    with (
        tc.tile_pool(name="sc_ps", bufs=2, space="PSUM") as sc_ps,
        tc.tile_pool(name="ctx_ps", bufs=1, space="PSUM") as ctx_ps,
        tc.tile_pool(name="flex_ps", bufs=1, space="PSUM") as flex_ps,
        tc.tile_pool(name="pt_pool", bufs=22) as pt_pool,
        tc.tile_pool(name="g_pool", bufs=2) as g_pool,
        tc.tile_pool(name="nrm_pool", bufs=2) as nrm_pool,
        tc.tile_pool(name="out_pool", bufs=3) as out_pool,
    ):
        def sc_tile(name="sc"):
            return sc_ps.tile([128, CH], f32, tag="sc", name=name)

        def flex_tile(shape, dtype, name):
            return flex_ps.tile(shape, dtype, tag="flex", name=name)

        def qkv_mm(pss, wi, cols0, dts):
            for dt_ in dts:
                for nn in range(2):
                    nc.tensor.matmul(
                        pss[:, nn * 512:(nn + 1) * 512],
                        lhsT=w_sb[wi][:, dt_, :],
                        rhs=xt_g[dt_][:, cols0 + nn * 512:cols0 + (nn + 1) * 512],
                        start=(dt_ == 0),
                        stop=(dt_ == ND - 1),
                    )

        def k_copy(pss, c0, hf):
            # one 512-col half: kT2[0] rows on Act (idle pre-exp), kT2[1]
            # rows on DVE; halves emitted low-cols-first across both tiles
            cols = slice(c0 + hf * 512, c0 + (hf + 1) * 512)
            nc.scalar.copy(kT2_sb[0][0:HD, cols],
                           pss[0:HD, hf * 512:(hf + 1) * 512])
            nc.vector.tensor_copy(kT2_sb[1][HD:128, cols],
                                  pss[HD:128, hf * 512:(hf + 1) * 512])

        # ---- k (both halves) + q chunk-0, interleaved per d-tile ----
        kp = [sc_tile("k0"), sc_tile("k1")]
        q0 = flex_tile([128, CH], f32, "q0")
        for dt_ in range(ND):
            for ti in range(2):
                qkv_mm(kp[ti], 0, ti * CH, [dt_])
            qkv_mm(q0, 2, 0, [dt_])
        # q0 copies first: the first scores tile needs ALL of qT_c[0] but
        # only kT2 cols 0:128, so q0 must not queue behind 4 k copies
        nc.scalar.copy(qT_c[0][:, 0:512], q0[:, 0:512])
        nc.vector.tensor_copy(qT_c[0][:, 512:CH], q0[:, 512:CH])
        for hf in range(2):
            for ti in range(2):
                k_copy(kp[ti], ti * CH, hf)

        # ---- deferred front work run inside attention(0,0)'s loop; flex
        # sequence: q0, v0, v1, q1, transposes, ctx(0,1), proj(0), ctx(1,1)
        vp_tiles = [None, None]

        def v_mm(ti, dts):
            if vp_tiles[ti] is None:
                vp_tiles[ti] = flex_tile([128, CH], f32, f"v{ti}")
            qkv_mm(vp_tiles[ti], 1, ti * CH, dts)

        def v_copy(ti):
            nc.vector.tensor_copy(
                vT_sb[:, ti * CH:(ti + 1) * CH], vp_tiles[ti][:]
            )

        q1_tile = [None]

        def q1_mm(dts):
            if q1_tile[0] is None:
                q1_tile[0] = flex_tile([128, CH], f32, "q1")
            qkv_mm(q1_tile[0], 2, CH, dts)

        def transposes(tts):
            for tt in tts:
                tp = flex_tile([128, 128], f16, "tr")
                nc.tensor.transpose(
                    tp[:], vT_sb[:, tt * 128:(tt + 1) * 128], ident_sb[:]
                )
                # one strided copy for both heads' 64 columns (the ones
                # columns at HD and 2HD+1 are skipped by the 65-stride)
                nc.vector.tensor_copy(
                    vp_sb[:, tt].rearrange("p (h c) -> p h c", h=2)[:, :, 0:HD],
                    tp[:].rearrange("p (h c) -> p h c", h=2),
                )

        front_hooks = {
            1: lambda: v_mm(0, range(0, 4)),
            2: lambda: (v_mm(0, range(4, 8)), v_copy(0)),
            3: lambda: v_mm(1, range(0, 4)),
            4: lambda: (v_mm(1, range(4, 8)), v_copy(1)),
            6: lambda: q1_mm(range(0, 4)),
            7: lambda: q1_mm(range(4, 8)),
            8: lambda: nc.vector.tensor_copy(qT_c[1][:], q1_tile[0][:]),
            9: lambda: transposes(range(0, 4)),
            10: lambda: transposes(range(4, 8)),
            11: lambda: transposes(range(8, 12)),
            12: lambda: transposes(range(12, 16)),
        }

        def attn(ci, h, cpool, ctag, pts_out=None, interleave=None,
                 dve_tts=()):
            """Scores+exp for NT t-tiles; ctx software-pipelined (tt-2)
            unless deferred via pts_out. Tiles in dve_tts compute exp on
            the DVE (poly + pow16 custom ops) instead of Activation."""
            cx = cpool.tile([HD + 1, CH], f32, tag=ctag, name="ctx")

            def ctx_mm(tt, pt):
                for nn in range(2):
                    nc.tensor.matmul(
                        cx[:, nn * 512:(nn + 1) * 512],
                        lhsT=vp_sb[:, tt, h * (HD + 1):(h + 1) * (HD + 1)],
                        rhs=pt[:, nn * 512:(nn + 1) * 512],
                        start=(tt == 0),
                        stop=(tt == NT - 1),
                    )

            pts = {}
            for tt in range(NT):
                sc = sc_ps.tile([128, CH], f32, tag="sc", name="sc")
                for nn in range(2):
                    nc.tensor.matmul(
                        sc[:, nn * 512:(nn + 1) * 512],
                        lhsT=kT2_sb[h][:, tt * 128:(tt + 1) * 128],
                        rhs=qT_c[ci][:, nn * 512:(nn + 1) * 512],
                        start=True,
                        stop=True,
                    )
                pt = pt_pool.tile([128, CH], f16, tag="pt")
                if tt in dve_tts:
                    g = g_pool.tile([128, CH], f16, tag="g", name="g")
                    nc.vector._custom_dve(op_poly, out=g[:], in0=sc[:],
                                          s0=PA1, s1=PA2, imm2=PA3)
                    nc.vector._custom_dve(op_pow16, out=pt[:], in0=g[:],
                                          s0=1.0)
                else:
                    nc.scalar.activation(pt[:], sc[:], Exp, scale=EXP_SCALE)
                if pts_out is not None:
                    pts_out.append(pt)
                else:
                    pts[tt] = pt
                    if tt >= 2:
                        ctx_mm(tt - 2, pts.pop(tt - 2))
                if interleave is not None and tt in interleave:
                    interleave[tt]()
            if pts_out is None:
                ctx_mm(NT - 2, pts.pop(NT - 2))
                ctx_mm(NT - 1, pts.pop(NT - 1))
            return cx, ctx_mm

        def norm(ci, h, soff, W, cx):
            # the custom DVE recip cannot read PSUM (BITWISE_NOT seed reads
            # garbage through the PSUM port) -- copy the denominator first
            den = nrm_pool.tile([1, W], f32, tag=f"den{W}", name="den")
            nc.vector.tensor_copy(den[:], cx[HD:HD + 1, soff:soff + W])
            rec = nrm_pool.tile([1, W], f32, tag=f"rec{W}", name="rec")
            nc.vector.reciprocal_approx_fast(rec[:], den[:])
            rb = nrm_pool.tile([HD, W], f32, tag=f"rb{W}", name="rb")
            nc.gpsimd.partition_broadcast(rb[:], rec[:])
            nc.vector.tensor_mul(
                ctxn_sb[h * HD:(h + 1) * HD, ci * CH + soff:ci * CH + soff + W],
                cx[0:HD, soff:soff + W],
                rb[:],
            )

        def proj_block(ci, b, cast_engs, pool):
            # cast in 512-halves right after each matmul half; with subtile
            # dep tracking the next block's first matmul only WARs the
            # first half-cast, halving the psum-ring serialization
            ob = out_pool.tile([128, D], f16, tag="ob", name="ob")
            if pool is sc_ps:
                ps = sc_tile("proj")
            elif pool is ctx_ps:
                ps = ctx_ps.tile([128, CH], f32, tag="ctx", name="proj")
            else:
                ps = flex_tile([128, CH], f32, "proj")
            for nn in range(2):
                nc.tensor.matmul(
                    ps[:, nn * 512:(nn + 1) * 512],
                    lhsT=ctxn_sb[:, ci * CH + b * 128:
                                 ci * CH + (b + 1) * 128],
                    rhs=wol_sb[:, nn * 512:(nn + 1) * 512],
                    start=True,
                    stop=True,
                )
            if cast_engs == 0:
                nc.scalar.copy(ob[:], ps[:])
            else:
                nc.vector.tensor_copy(ob[:], ps[:])
            nc.sync.dma_start(out[ci, b], ob[:])

        # (0,0): scores+exp only, front work interleaved, pts kept
        pts00 = []
        cx00, ctx00_mm = attn(0, 0, ctx_ps, "ctx",
                              pts_out=pts00, interleave=front_hooks)
        # (0,1): ctx(0,0) at 2 tiles/step in the first half so norm(0,0)
        # lands mid-window and (1,0)'s ctx psum frees early
        cx01, _ = attn(0, 1, flex_ps, "flex",
                       interleave={tt: (lambda tt=tt: (
                           ctx00_mm(2 * tt, pts00[2 * tt]),
                           ctx00_mm(2 * tt + 1, pts00[2 * tt + 1]),
                           norm(0, 0, 0, CH, cx00) if tt == 7 else None,
                       )) for tt in range(8)})
        # (1,0): norm(0,1) emitted at tt2 (late: keeps its DVE ops out of
        # the exp-critical DVE queue head), proj(0) blocks from tt4
        h10 = {2: lambda: norm(0, 1, 0, CH, cx01)}
        for i in range(NB):
            h10[4 + i] = (lambda i=i: proj_block(0, i, 1, flex_ps))
        cx10, _ = attn(1, 0, ctx_ps, "ctx", interleave=h10, dve_tts=(13,))
        # (1,1): DVE absorbs 4 exp tiles (Act sustains only ~1.1-1.3us per
        # exp tile under its activity throttle); norm(1,0) is consumed
        # only by the chunk-1 projections, so it emits late at tt12
        cx11, _ = attn(1, 1, flex_ps, "flex", dve_tts=(2, 6, 10, 13),
                       interleave={12: lambda: norm(1, 0, 0, CH, cx10)})
        # clock bridge over the tail normalize window
        for i in range(2):
            jp = sc_tile("junk")
            nc.tensor.matmul(
                jp[:, 0:512], lhsT=junk_sb[:, 0:128],
                rhs=junk_sb[:, 0:512], start=True, stop=True,
            )
        # tail: norm in 512 halves (chains interleaved), 8 proj blocks
        # through a 3-psum ring (sc, sc, flex), casts split Act/DVE per
        # block half (Act is idle after the last exp)
        norm(1, 1, 0, 512, cx11)
        norm(1, 1, 512, 512, cx11)
        for b in range(NB):
            pool = (sc_ps, sc_ps, flex_ps, ctx_ps)[b % 4]
            proj_block(1, b, b % 2, pool)

    ctx_stack.close()


def get_nc(enable_asserts=False):
    key = ("nc", enable_asserts)
    if key not in _CACHE:
        _CACHE[key] = _build(enable_asserts)
    return _CACHE[key]


def make_in_maps(x, w_in, w_out):
    x = np.asarray(x, dtype=np.float32)
    w_in = np.asarray(w_in, dtype=np.float32)
    w_out = np.asarray(w_out, dtype=np.float32)
    xT = np.ascontiguousarray(x.T).astype(np.float16).reshape(ND, 128, S)
    w_outT = np.ascontiguousarray(w_out.T).astype(np.float16)  # [A, D]
    in_maps = []
    for c in range(NCORES):
        r0 = c * E
        def pmaj(wt, scale=1.0):  # [D, E] -> p-major [128, ND*E]
            return np.ascontiguousarray(
                (wt * scale).reshape(ND, 128, E).transpose(1, 0, 2)
                .reshape(128, ND * E)
            ).astype(np.float16)
        wq = pmaj(w_in[r0:r0 + E].T)
        # kT2 prescale: scores psum = qk/128 = s_hat/16 (see EXP_SCALE)
        wk = pmaj(w_in[A + r0:A + r0 + E].T, scale=1.0 / 128.0)
        wv = pmaj(w_in[2 * A + r0:2 * A + r0 + E].T)
        wol = np.ascontiguousarray(w_outT[r0:r0 + E])  # [128, D]
        in_maps.append({"xT": xT, "wk": wk, "wv": wv, "wq": wq, "wol": wol})
    return in_maps


def assemble_out(results):
    """results[c]["out"] is [NCH, NB, 128, D] fp16 partials in s-block
    order; the unshard step sums the 8 cores' partial projections."""
    full = np.zeros((S, D), dtype=np.float32)
    for c in range(NCORES):
        o = results[c]["out"].astype(np.float32).reshape(S, D)
        full += o
    return full


def kernel(x, w_in, w_out, tgt_len=None, **kwargs):
    from concourse.bass_utils import run_bass_kernel_spmd

    nc = get_nc()
    in_maps = make_in_maps(x, w_in, w_out)
    res = run_bass_kernel_spmd(nc, in_maps, core_ids=list(range(NCORES)))
    return assemble_out(res.results)


# revision 21
# speedup vs baseline: 1.0407x; 1.0192x over previous
"""Multi-headed self-attention (S=2048, D=1024, H=16) on 8 trn2 NeuronCores.

Sharding: tensor-parallel over heads (2 heads/core), fully collective-free.
Each core computes q/k/v for its 2 heads, runs base-2 no-max softmax
attention, and then computes the PARTIAL output projection for the full
[S, D] output (K-split over heads). The host unshard step sums the 8
partial outputs.

v3 (from HW trace analysis of v2 @156.8us):
- Pass order k -> q(chunk0) -> attention; v-pass, q(chunk1) and the v'
  transposes interleave into attention(0,0)'s tt loop (q1 BEFORE the
  transposes: in v2 the in-order PE queue ran q1 ~15us late and stalled
  chunk-1 attention for 5.5us). ctx(0,0) defers into (0,1)'s loop,
  2 t-tiles per step so norm(0,0) lands mid-(0,1).
- The exp softmax is SPLIT across engines: the Activation engine
  sustains only ~1.33us per [128,1024] exp tile under its activity
  throttle, so in the windows where it is the bottleneck a subset of
  t-tiles is computed on the DVE via two custom table ops:
  g = t(a1+t(a2+t a3)) ~= 2^t - 1 (scores prescaled by 1/128 via kT2),
  then p = (1+g)^16. Max rel err 4.5e-3 (measured on HW), fro impact
  ~2e-4 -- the scores PSUM stays fp32, only p is fp16.
- Tail: single full-width (1,1) pass (v2's 512-split cost +6us of
  Activation time); normalize runs in two 512 halves and the 8 chunk-1
  projection blocks pipeline through a 3-psum ring (sc ring + flex)
  with casts alternating Activation/DVE (Activation is idle post-exp).
- PSUM: sc pool 2x[128,1024]f32, ctx pool 1x, flex pool 1x shared (in
  strict sequence) by q0/v0/v1/q1 qkv psums, v' transposes, ctx(0,1),
  proj(0) blocks, ctx(1,1), and every third tail proj block.
"""

import sys

import numpy as np

if "/opt/trn_rl_repo" not in sys.path:
    sys.path.insert(0, "/opt/trn_rl_repo")

S, D, A, H = 2048, 1024, 1024, 16
NCORES = 8
HPC = H // NCORES            # heads per core = 2
HD = A // H                  # head dim = 64
E = HPC * HD                 # local qkv rows = 128
ND = D // 128                # d tiles = 8
NT = S // 128                # t tiles = 16
LN2 = 0.6931471805599453
# kT2 is prescaled by 1/128 host-side => scores psum = qk/128 = s_hat/16
# Act path: p = exp(psum * 16 ln2) = 2^s_hat;  DVE path: p = (1+g(t))^16
EXP_SCALE = 16.0 * LN2

# minimax-ish fit of g(t) ~= 2^t - 1 on t in [-0.62, 0.62], weighted for
# relative error of (1+g)^16 (see fit in the build notes); max rel err of
# the composed p on |s_hat|<=9.9 is ~6.6e-3 in fp32, 4.5e-3 measured.
PA1, PA2, PA3 = 0.6935366256724811, 0.24282106648173085, 0.05415638145524527

NCH = 2                      # attention s-chunks
CH = S // NCH                # 1024
NB = CH // 128               # proj s-blocks per chunk = 8

_CACHE = {}


def _register_dve_ops():
    """Register the two custom DVE table ops used by the DVE exp path.
    Idempotent; sha computed at runtime against this container's lower()."""
    from concourse import dve_ops
    from concourse.dve_spec import (
        Spec, Src0, C0, C1, C2, lower, _has_src1 as has_src1, sq,
    )
    from concourse.dve_uop import DveOpSpec
    from concourse.dve_table_gen import dve_ver_for

    have = {op.name for op in dve_ops.OPS}

    def register(name, body, ref):
        if name in have:
            return next(op for op in dve_ops.OPS if op.name == name)
        ver = dve_ver_for("TRN2")
        sp = Spec(body=body, reference=ref)
        row = dve_ops._CUSTOM_DVE_ROW_BASE + len(dve_ops.OPS)
        probe = DveOpSpec(name=name, opcode=row, uops=lower(sp, ver=ver),
                          rd1_en=has_src1(sp))
        op = dve_ops.DveOp(name, sp, subdim=False,
                           uops_sha={ver: probe.sha(ver)})
        dve_ops.OPS.append(op)
        dve_ops.CUSTOM_DVE_SPECS[name] = sp
        dve_ops._SUB_OPCODE_FOR_NAME[name] = row
        return op

    op_poly = register(
        "EXP2G_POLY_ANT",
        Src0 * (C0 + Src0 * (C1 + Src0 * C2)),
        lambda in0, in1, s0, s1, imm2: (
            in0.astype(np.float32) * (s0 + in0 * (s1 + in0 * imm2))),
    )
    op_pow16 = register(
        "EXP2G_POW16_ANT",
        sq(sq(sq(sq(Src0 + C0)))),
        lambda in0, in1, s0, s1, imm2: ((in0.astype(np.float32) + s0) ** 16),
    )
    return op_poly, op_pow16


def _build(enable_asserts=False):
    import concourse.bass as bass
    import concourse.tile as tile
    import concourse.mybir as mybir
    from concourse import bacc
    from concourse.masks import make_identity

    dve_exp_ops = _register_dve_ops()

    f16 = mybir.dt.float16

    nc = bacc.Bacc(
        "TRN2",
        target_bir_lowering=False,
        debug=False,
        enable_asserts=enable_asserts,
        num_devices=NCORES,
    )

    xT = nc.dram_tensor("xT", [ND, 128, S], f16, kind="ExternalInput").ap()
    wk = nc.dram_tensor("wk", [128, ND * E], f16, kind="ExternalInput").ap()
    wv = nc.dram_tensor("wv", [128, ND * E], f16, kind="ExternalInput").ap()
    wq = nc.dram_tensor("wq", [128, ND * E], f16, kind="ExternalInput").ap()
    wol = nc.dram_tensor("wol", [128, D], f16, kind="ExternalInput").ap()
    out = nc.dram_tensor("out", [NCH, NB, 128, D], f16, kind="ExternalOutput").ap()

    with tile.TileContext(nc) as tc:
        _body(tc, xT, (wk, wv, wq), wol, out, mybir, bass, make_identity,
              dve_exp_ops)

    nc.compile()
    return nc


def _body(tc, xT, wkvq, wol, out, mybir, bass, make_identity, dve_exp_ops):
    from contextlib import ExitStack

    nc = tc.nc
    f16 = mybir.dt.float16
    f32 = mybir.dt.float32
    Exp = mybir.ActivationFunctionType.Exp
    op_poly, op_pow16 = dve_exp_ops

    ctx_stack = ExitStack()
    persist = ctx_stack.enter_context(tc.tile_pool(name="persist", bufs=1))

    def ptile(shape, dtype, name):
        return persist.tile(shape, dtype, tag=name, name=name)

    xt_g = [ptile([128, S], f16, f"xt_g{g}") for g in range(ND)]
    w_sb = [ptile([128, ND, E], f16, f"w_sb{i}") for i in range(3)]  # k,v,q
    wol_sb = ptile([128, D], f16, "wol_sb")
    qT_c = [ptile([128, CH], f16, f"qT_c{ci}") for ci in range(NCH)]
    kT2_sb = [ptile([128, S], f16, f"kT2_sb{h}") for h in range(HPC)]
    vT_sb = ptile([128, S], f16, "vT_sb")
    vp_sb = ptile([128, NT, 2 * (HD + 1)], f16, "vp_sb")
    ident_sb = ptile([128, 128], f16, "ident_sb")
    ctxn_sb = ptile([128, S], f16, "ctxn_sb")
    junk_sb = ptile([128, 512], f16, "junk_sb")

    nc.vector.memset(kT2_sb[0][HD:128, :], 0.0)
    nc.vector.memset(kT2_sb[1][0:HD, :], 0.0)
    nc.vector.memset(vp_sb[:, :, HD:HD + 1], 1.0)
    nc.vector.memset(vp_sb[:, :, 2 * HD + 1:2 * HD + 2], 1.0)
    nc.vector.memset(junk_sb[:], 0.0)
    make_identity(nc, ident_sb[:])

    nc.sync.dma_start(w_sb[0][:], wkvq[0].rearrange("p (t c) -> p t c", t=ND))
    for g in range(ND):
        eng = nc.scalar if g % 2 == 0 else nc.sync
        eng.dma_start(xt_g[g][:], xT[g])
    nc.scalar.dma_start(w_sb[1][:], wkvq[1].rearrange("p (t c) -> p t c", t=ND))
    nc.scalar.dma_start(w_sb[2][:], wkvq[2].rearrange("p (t c) -> p t c", t=ND))
    nc.scalar.dma_start(wol_sb[:], wol)

    with (
        tc.tile_pool(name="sc_ps", bufs=2, space="PSUM") as sc_ps,
        tc.tile_pool(name="ctx_ps", bufs=1, space="PSUM") as ctx_ps,
        tc.tile_pool(name="flex_ps", bufs=1, space="PSUM") as flex_ps,
        tc.tile_pool(name="pt_pool", bufs=22) as pt_pool,
        tc.tile_pool(name="g_pool", bufs=2) as g_pool,
        tc.tile_pool(name="nrm_pool", bufs=2) as nrm_pool,
        tc.tile_pool(name="out_pool", bufs=4) as out_pool,
    ):
        def sc_tile(name="sc"):
            return sc_ps.tile([128, CH], f32, tag="sc", name=name)

        def flex_tile(shape, dtype, name):
            return flex_ps.tile(shape, dtype, tag="flex", name=name)

        def qkv_mm(pss, wi, cols0, dts):
            for dt_ in dts:
                for nn in range(2):
                    nc.tensor.matmul(
                        pss[:, nn * 512:(nn + 1) * 512],
                        lhsT=w_sb[wi][:, dt_, :],
                        rhs=xt_g[dt_][:, cols0 + nn * 512:cols0 + (nn + 1) * 512],
                        start=(dt_ == 0),
                        stop=(dt_ == ND - 1),
                    )

        def k_copy(pss, c0, hf):
            # one 512-col half: kT2[0] rows on Act (idle pre-exp), kT2[1]
            # rows on DVE; halves emitted low-cols-first across both tiles
            cols = slice(c0 + hf * 512, c0 + (hf + 1) * 512)
            nc.scalar.copy(kT2_sb[0][0:HD, cols],
                           pss[0:HD, hf * 512:(hf + 1) * 512])
            nc.vector.tensor_copy(kT2_sb[1][HD:128, cols],
                                  pss[HD:128, hf * 512:(hf + 1) * 512])

        # ---- k (both halves) + q chunk-0, interleaved per d-tile ----
        kp = [sc_tile("k0"), sc_tile("k1")]
        q0 = flex_tile([128, CH], f32, "q0")
        for dt_ in range(ND):
            for ti in range(2):
                qkv_mm(kp[ti], 0, ti * CH, [dt_])
            qkv_mm(q0, 2, 0, [dt_])
        # q0 copies first: the first scores tile needs ALL of qT_c[0] but
        # only kT2 cols 0:128, so q0 must not queue behind 4 k copies
        nc.scalar.copy(qT_c[0][:, 0:512], q0[:, 0:512])
        nc.vector.tensor_copy(qT_c[0][:, 512:CH], q0[:, 512:CH])
        for hf in range(2):
            for ti in range(2):
                k_copy(kp[ti], ti * CH, hf)

        # ---- deferred front work run inside attention(0,0)'s loop; flex
        # sequence: q0, v0, v1, q1, transposes, ctx(0,1), proj(0), ctx(1,1)
        vp_tiles = [None, None]

        def v_mm(ti, dts):
            if vp_tiles[ti] is None:
                vp_tiles[ti] = flex_tile([128, CH], f32, f"v{ti}")
            qkv_mm(vp_tiles[ti], 1, ti * CH, dts)

        def v_copy(ti):
            nc.vector.tensor_copy(
                vT_sb[:, ti * CH:(ti + 1) * CH], vp_tiles[ti][:]
            )

        q1_tile = [None]

        def q1_mm(dts):
            if q1_tile[0] is None:
                q1_tile[0] = flex_tile([128, CH], f32, "q1")
            qkv_mm(q1_tile[0], 2, CH, dts)

        def transposes(tts):
            for tt in tts:
                tp = flex_tile([128, 128], f16, "tr")
                nc.tensor.transpose(
                    tp[:], vT_sb[:, tt * 128:(tt + 1) * 128], ident_sb[:]
                )
                # one strided copy for both heads' 64 columns (the ones
                # columns at HD and 2HD+1 are skipped by the 65-stride)
                nc.vector.tensor_copy(
                    vp_sb[:, tt].rearrange("p (h c) -> p h c", h=2)[:, :, 0:HD],
                    tp[:].rearrange("p (h c) -> p h c", h=2),
                )

        front_hooks = {
            1: lambda: v_mm(0, range(0, 4)),
            2: lambda: (v_mm(0, range(4, 8)), v_copy(0)),
            3: lambda: v_mm(1, range(0, 4)),
            4: lambda: (v_mm(1, range(4, 8)), v_copy(1)),
            6: lambda: q1_mm(range(0, 4)),
            7: lambda: q1_mm(range(4, 8)),
            8: lambda: nc.vector.tensor_copy(qT_c[1][:], q1_tile[0][:]),
            9: lambda: transposes(range(0, 4)),
            10: lambda: transposes(range(4, 8)),
            11: lambda: transposes(range(8, 12)),
            12: lambda: transposes(range(12, 16)),
        }

        def attn(ci, h, cpool, ctag, pts_out=None, interleave=None,
                 dve_tts=()):
            """Scores+exp for NT t-tiles; ctx software-pipelined (tt-2)
            unless deferred via pts_out. Tiles in dve_tts compute exp on
            the DVE (poly + pow16 custom ops) instead of Activation."""
            cx = cpool.tile([HD + 1, CH], f32, tag=ctag, name="ctx")

            def ctx_mm(tt, pt):
                for nn in range(2):
                    nc.tensor.matmul(
                        cx[:, nn * 512:(nn + 1) * 512],
                        lhsT=vp_sb[:, tt, h * (HD + 1):(h + 1) * (HD + 1)],
                        rhs=pt[:, nn * 512:(nn + 1) * 512],
                        start=(tt == 0),
                        stop=(tt == NT - 1),
                    )

            pts = {}
            for tt in range(NT):
                sc = sc_ps.tile([128, CH], f32, tag="sc", name="sc")
                for nn in range(2):
                    nc.tensor.matmul(
                        sc[:, nn * 512:(nn + 1) * 512],
                        lhsT=kT2_sb[h][:, tt * 128:(tt + 1) * 128],
                        rhs=qT_c[ci][:, nn * 512:(nn + 1) * 512],
                        start=True,
                        stop=True,
                    )
                pt = pt_pool.tile([128, CH], f16, tag="pt")
                if tt in dve_tts:
                    g = g_pool.tile([128, CH], f16, tag="g", name="g")
                    nc.vector._custom_dve(op_poly, out=g[:], in0=sc[:],
                                          s0=PA1, s1=PA2, imm2=PA3)
                    nc.vector._custom_dve(op_pow16, out=pt[:], in0=g[:],
                                          s0=1.0)
                else:
                    nc.scalar.activation(pt[:], sc[:], Exp, scale=EXP_SCALE)
                if pts_out is not None:
                    pts_out.append(pt)
                else:
                    pts[tt] = pt
                    if tt >= 2:
                        ctx_mm(tt - 2, pts.pop(tt - 2))
                if interleave is not None and tt in interleave:
                    interleave[tt]()
            if pts_out is None:
                ctx_mm(NT - 2, pts.pop(NT - 2))
                ctx_mm(NT - 1, pts.pop(NT - 1))
            return cx, ctx_mm

        def norm(ci, h, soff, W, cx, den_eng=1):
            # the custom DVE recip cannot read PSUM (BITWISE_NOT seed reads
            # garbage through the PSUM port) -- copy the denominator first
            den = nrm_pool.tile([1, W], f32, tag=f"den{W}", name="den")
            if den_eng == 0:
                nc.scalar.copy(den[:], cx[HD:HD + 1, soff:soff + W])
            else:
                nc.vector.tensor_copy(den[:], cx[HD:HD + 1, soff:soff + W])
            rec = nrm_pool.tile([1, W], f32, tag=f"rec{W}", name="rec")
            nc.vector.reciprocal_approx_fast(rec[:], den[:])
            rb = nrm_pool.tile([HD, W], f32, tag=f"rb{W}", name="rb")
            nc.gpsimd.partition_broadcast(rb[:], rec[:])
            nc.vector.tensor_mul(
                ctxn_sb[h * HD:(h + 1) * HD, ci * CH + soff:ci * CH + soff + W],
                cx[0:HD, soff:soff + W],
                rb[:],
            )

        def proj_block(ci, b, cast_engs, pool):
            # cast in 512-halves right after each matmul half; with subtile
            # dep tracking the next block's first matmul only WARs the
            # first half-cast, halving the psum-ring serialization
            ob = out_pool.tile([128, D], f16, tag="ob", name="ob")
            if pool is sc_ps:
                ps = sc_tile("proj")
            elif pool is ctx_ps:
                ps = ctx_ps.tile([128, CH], f32, tag="ctx", name="proj")
            else:
                ps = flex_tile([128, CH], f32, "proj")
            for nn in range(2):
                nc.tensor.matmul(
                    ps[:, nn * 512:(nn + 1) * 512],
                    lhsT=ctxn_sb[:, ci * CH + b * 128:
                                 ci * CH + (b + 1) * 128],
                    rhs=wol_sb[:, nn * 512:(nn + 1) * 512],
                    start=True,
                    stop=True,
                )
            if cast_engs == 0:
                nc.scalar.copy(ob[:], ps[:])
            else:
                nc.vector.tensor_copy(ob[:], ps[:])
            nc.sync.dma_start(out[ci, b], ob[:])

        # (0,0): scores+exp only, front work interleaved, pts kept
        pts00 = []
        cx00, ctx00_mm = attn(0, 0, ctx_ps, "ctx",
                              pts_out=pts00, interleave=front_hooks)
        # (0,1): ctx(0,0) at 2 tiles/step in the first half so norm(0,0)
        # lands mid-window and (1,0)'s ctx psum frees early
        cx01, _ = attn(0, 1, flex_ps, "flex",
                       interleave={tt: (lambda tt=tt: (
                           ctx00_mm(2 * tt, pts00[2 * tt]),
                           ctx00_mm(2 * tt + 1, pts00[2 * tt + 1]),
                           norm(0, 0, 0, CH, cx00) if tt == 7 else None,
                       )) for tt in range(8)})
        # (1,0): norm(0,1) emitted at tt2 (late: keeps its DVE ops out of
        # the exp-critical DVE queue head), proj(0) blocks from tt4
        h10 = {2: lambda: norm(0, 1, 0, CH, cx01)}
        for i in range(NB):
            h10[4 + i] = (lambda i=i: proj_block(0, i, 1, flex_ps))
        cx10, _ = attn(1, 0, ctx_ps, "ctx", interleave=h10)
        # (1,1): DVE absorbs 4 exp tiles (Act sustains only ~1.1-1.3us per
        # exp tile under its activity throttle); norm(1,0) is consumed
        # only by the chunk-1 projections, so it emits late at tt12
        cx11, _ = attn(1, 1, flex_ps, "flex", dve_tts=(2, 6, 10, 13),
                       interleave={12: lambda: norm(1, 0, 0, CH, cx10)})
        # clock bridge over the tail normalize window
        for i in range(2):
            jp = sc_tile("junk")
            nc.tensor.matmul(
                jp[:, 0:512], lhsT=junk_sb[:, 0:128],
                rhs=junk_sb[:, 0:512], start=True, stop=True,
            )
        # tail: norm in 512 halves (chains interleaved), 8 proj blocks
        # through a 3-psum ring (sc, sc, flex), casts split Act/DVE per
        # block half (Act is idle after the last exp)
        norm(1, 1, 0, 512, cx11, den_eng=0)
        norm(1, 1, 512, 512, cx11, den_eng=0)
        for b in range(NB):
            pool = (sc_ps, sc_ps, flex_ps, ctx_ps)[b % 4]
            proj_block(1, b, b % 2, pool)

    ctx_stack.close()


def get_nc(enable_asserts=False):
    key = ("nc", enable_asserts)
    if key not in _CACHE:
        _CACHE[key] = _build(enable_asserts)
    return _CACHE[key]


def make_in_maps(x, w_in, w_out):
    x = np.asarray(x, dtype=np.float32)
    w_in = np.asarray(w_in, dtype=np.float32)
    w_out = np.asarray(w_out, dtype=np.float32)
    xT = np.ascontiguousarray(x.T).astype(np.float16).reshape(ND, 128, S)
    w_outT = np.ascontiguousarray(w_out.T).astype(np.float16)  # [A, D]
    in_maps = []
    for c in range(NCORES):
        r0 = c * E
        def pmaj(wt, scale=1.0):  # [D, E] -> p-major [128, ND*E]
            return np.ascontiguousarray(
                (wt * scale).reshape(ND, 128, E).transpose(1, 0, 2)
                .reshape(128, ND * E)
            ).astype(np.float16)
        wq = pmaj(w_in[r0:r0 + E].T)
        # kT2 prescale: scores psum = qk/128 = s_hat/16 (see EXP_SCALE)
        wk = pmaj(w_in[A + r0:A + r0 + E].T, scale=1.0 / 128.0)
        wv = pmaj(w_in[2 * A + r0:2 * A + r0 + E].T)
        wol = np.ascontiguousarray(w_outT[r0:r0 + E])  # [128, D]
        in_maps.append({"xT": xT, "wk": wk, "wv": wv, "wq": wq, "wol": wol})
    return in_maps


def assemble_out(results):
    """results[c]["out"] is [NCH, NB, 128, D] fp16 partials in s-block
    order; the unshard step sums the 8 cores' partial projections."""
    full = np.zeros((S, D), dtype=np.float32)
    for c in range(NCORES):
        o = results[c]["out"].astype(np.float32).reshape(S, D)
        full += o
    return full


def kernel(x, w_in, w_out, tgt_len=None, **kwargs):
    from concourse.bass_utils import run_bass_kernel_spmd

    nc = get_nc()
    in_maps = make_in_maps(x, w_in, w_out)
    res = run_bass_kernel_spmd(nc, in_maps, core_ids=list(range(NCORES)))
    return assemble_out(res.results)
